# revision 25
# baseline (speedup 1.0000x reference)
"""Trainium2 Bass kernel for nn_Attention (additive-attention block).

Computation (B=256, Hde=E=Catt=512, S=Hf*Wf=256):
    g     = h @ Wh.T + bh                      [B, E]
    feat  = tanh(x_em + g[:, :, None, None])   [B, E, Hf, Wf]
    score = einsum('bes,e->bs', feat, Wa) + ba [B, S]   (ba cancels in softmax)
    alpha = softmax_S(score)                   [B, S]
    out   = einsum('bs,bsc->bc', alpha, att)   [B, Catt]
Returns (att_out [B, Catt], alpha [B, 1, S]).

Sharding: data-parallel over batch B across 8 NeuronCores (32 batches per
core), weights replicated; no collectives.  Gather on host.

The two big reductions run on the PE in float32r (fp32 bits, ~11-bit
mantissa multiply) which streams at 1 cycle/row for N>=256 vs 4 for fp32;
measured mini-test rel err 2.2e-4.
"""

import numpy as np

import concourse.bacc as bacc
import concourse.bass as bass  # noqa: F401
import concourse.mybir as mybir
import concourse.tile as tile
from concourse import masks
from concourse.bass_utils import run_bass_kernel_spmd

N_CORES = 8
B, Hde, E, Catt = 256, 512, 512, 512
S = 256  # Hf * Wf = 8 * 32
Bc = B // N_CORES  # 32 batches per core
P = 128
ET = E // P    # 4 tiles over E
DT = Hde // P  # 4 tiles over Hde
ST = S // P    # 2 tiles over S
F32 = mybir.dt.float32
F32R = mybir.dt.float32r
AFT = mybir.ActivationFunctionType
AX = mybir.AxisListType

USE_F32R = True

_CACHE = {}


GROUPS = (16, 16)  # batches per softmax/output group
assert sum(GROUPS) == Bc
G = max(GROUPS)


def _emit(nc, tc, hT_d, x_d, att_d, whT_d, bh_d, wa_d, out_d, alpha_d):
    FMM = F32R if USE_F32R else F32

    def mmcast(ap):
        return ap.bitcast(F32R) if USE_F32R else ap

    with (
        tc.tile_pool(name="const", bufs=1) as cpool,
        tc.tile_pool(name="xin", bufs=5) as xpool,
        tc.tile_pool(name="attres", bufs=Bc) as apool,
        tc.tile_pool(name="feat", bufs=4) as fpool,
        tc.tile_pool(name="grp", bufs=2) as gpool,
        tc.tile_pool(name="ps", bufs=2, space="PSUM") as pspool,
        tc.tile_pool(name="psrow", bufs=3, space="PSUM") as rowpool,
    ):
        # ---- constants ----
        # e-index mapping everywhere: e = 4*p + t  (contiguous 4KB per
        # partition on the x_em DMA); s-index mapping: s = 2*p + t.
        whT_sb = cpool.tile([P, DT, E], F32)
        whT_r = whT_d[:].rearrange("(t p) e -> p t e", p=P)
        for dt in range(DT):
            nc.sync.dma_start(whT_sb[:, dt, :], whT_r[:, dt, :])
        hT_sb = cpool.tile([P, DT, Bc], F32)
        nc.sync.dma_start(hT_sb[:], hT_d[:].rearrange("(t p) b -> p t b", p=P))
        bh_sb = cpool.tile([P, ET], F32)
        nc.gpsimd.dma_start(bh_sb[:], bh_d[:].rearrange("(p t) -> p t", p=P))
        wa_sb = cpool.tile([P, ET], FMM)
        nc.gpsimd.dma_start(wa_sb[:],
                            mmcast(wa_d[:].rearrange("(p t) -> p t", p=P)))
        ident = cpool.tile([P, P], F32)
        masks.make_identity(nc, ident[:])

        # ---- gg[p, t, b] = (Wh @ h.T)[4p+t, b] + bh[4p+t] ----
        gg_sb = cpool.tile([P, ET, Bc], F32)
        for t in range(ET):
            gt_ps = pspool.tile([P, Bc], F32, tag="t32")
            for dt in range(DT):
                nc.tensor.matmul(
                    gt_ps[:],
                    lhsT=whT_sb[:, dt, t::ET],
                    rhs=hT_sb[:, dt, :],
                    start=(dt == 0), stop=(dt == DT - 1),
                )
            nc.vector.tensor_scalar_add(gg_sb[:, t, :], gt_ps[:],
                                        bh_sb[:, t:t + 1])

        att_tiles = {}
        g_state = {}

        def phase_a(g):
            Gg = GROUPS[g]
            b0 = sum(GROUPS[:g])
            scores_g = gpool.tile([Gg, S], F32, tag="scores", name=f"scores_{g}")
            for j2 in range(Gg // 2):
                sc_ps = rowpool.tile([1, 2, S], F32, tag="sc", bufs=3,
                                     name=f"sc_ps_{g}_{j2}")
                for j in range(2):
                    b = b0 + j2 * 2 + j
                    x_sb = xpool.tile([P, ET, S], F32, tag="x", name=f"x_{b}")
                    nc.sync.dma_start(
                        x_sb[:], x_d[b].rearrange("(p t) s -> p t s", p=P))
                    att_sb = apool.tile([P, ST, Catt], FMM, tag="att",
                                        name=f"att_{b}")
                    nc.sync.dma_start(
                        att_sb[:],
                        mmcast(att_d[b].rearrange("(p t) c -> p t c", p=P)))
                    att_tiles[b] = att_sb
                    for t in range(ET):
                        feat_sb = fpool.tile([P, S], FMM, tag="feat",
                                             name=f"feat_{b}_{t}")
                        nc.scalar.activation(feat_sb[:], x_sb[:, t, :], AFT.Tanh,
                                             bias=gg_sb[:, t, b:b + 1], scale=1.0)
                        nc.tensor.matmul(
                            sc_ps[0:1, j, :],
                            lhsT=wa_sb[:, t:t + 1],
                            rhs=feat_sb[:],
                            start=(t == 0), stop=(t == ET - 1),
                        )
                srow_sb = fpool.tile([1, 2, S], F32, tag="srow", bufs=4,
                                     name=f"srow_{g}_{j2}")
                nc.vector.tensor_copy(srow_sb[:], sc_ps[:])
                nc.gpsimd.dma_start(scores_g[j2 * 2:(j2 + 1) * 2, :],
                                    srow_sb[:])
            g_state[g] = scores_g

        def phase_b(g):
            Gg = GROUPS[g]
            b0 = sum(GROUPS[:g])
            scores_g = g_state.pop(g)
            alpha_g = gpool.tile([Gg, S], F32, tag="alpha", name=f"alpha_{g}")
            ssum = gpool.tile([Gg, 1], F32, tag="ssum", name=f"ssum_{g}")
            nc.scalar.activation(alpha_g[:], scores_g[:], AFT.Exp,
                                 bias=0.0, scale=1.0, accum_out=ssum[:])
            rinv = gpool.tile([Gg, 1], F32, tag="rinv", name=f"rinv_{g}")
            nc.vector.reciprocal(rinv[:], ssum[:])
            nc.vector.tensor_scalar_mul(alpha_g[:], alpha_g[:], rinv[:])
            nc.gpsimd.dma_start(alpha_d[b0:b0 + Gg, :], alpha_g[:])

            alphaT_g = gpool.tile([P, ST, Gg], FMM, tag="alphaT",
                                  name=f"alphaT_{g}")
            for t in range(ST):
                tps = pspool.tile([P, Gg], F32, tag="t32", name=f"tps_{g}_{t}")
                nc.tensor.transpose(tps[:], alpha_g[:, t::ST], ident[:Gg, :Gg])
                nc.scalar.activation(alphaT_g[:, t, :], tps[:], AFT.Identity)

            for j2 in range(Gg // 2):
                aorow_sb = fpool.tile([1, 2, Catt], F32, tag="aorow", bufs=4,
                                      name=f"aorow_{g}_{j2}")
                for j in range(2):
                    b = b0 + j2 * 2 + j
                    ao_ps = rowpool.tile([1, Catt], F32, tag="ao", bufs=3,
                                         name=f"ao_ps_{b}")
                    for t in range(ST):
                        nc.tensor.matmul(
                            ao_ps[:],
                            lhsT=alphaT_g[:, t, b - b0:b - b0 + 1],
                            rhs=att_tiles[b][:, t, :],
                            start=(t == 0), stop=(t == ST - 1),
                        )
                    nc.vector.tensor_copy(aorow_sb[0:1, j, :], ao_ps[:])
                nc.gpsimd.dma_start(out_d[b0 + j2 * 2:b0 + (j2 + 1) * 2, :],
                                    aorow_sb[:])

        # software-pipelined emission: phase B of group g-1 is emitted inside
        # group g's phase A so the in-order PE/ACT streams never stall at a
        # group boundary.
        for g in range(len(GROUPS)):
            phase_a(g)
            if g >= 1:
                phase_b(g - 1)
        phase_b(len(GROUPS) - 1)


def _build(reps=1):
    nc = bacc.Bacc("TRN2", target_bir_lowering=False, debug=False,
                   num_devices=N_CORES)

    hT_d = nc.dram_tensor("hT", [Hde, Bc], F32, kind="ExternalInput")
    x_d = nc.dram_tensor("x_em", [Bc, E, S], F32, kind="ExternalInput")
    att_d = nc.dram_tensor("att_x_em", [Bc, S, Catt], F32, kind="ExternalInput")
    whT_d = nc.dram_tensor("WhT", [Hde, E], F32, kind="ExternalInput")
    bh_d = nc.dram_tensor("bh", [E], F32, kind="ExternalInput")
    wa_d = nc.dram_tensor("Wa", [E], F32, kind="ExternalInput")
    out_d = nc.dram_tensor("att_out", [Bc, Catt], F32, kind="ExternalOutput")
    alpha_d = nc.dram_tensor("alpha", [Bc, S], F32, kind="ExternalOutput")

    with tile.TileContext(nc) as tc:
        for _rep in range(reps):
            _emit(nc, tc, hT_d, x_d, att_d, whT_d, bh_d, wa_d, out_d, alpha_d)

    nc.compile()
    return nc


def _get_nc(reps=1):
    key = ("nc", reps)
    if key not in _CACHE:
        _CACHE[key] = _build(reps)
    return _CACHE[key]


def _in_maps(h, x_em, att_x_em, Wh, bh, Wa):
    h = np.asarray(h, dtype=np.float32)
    x_em = np.asarray(x_em, dtype=np.float32).reshape(B, E, S)
    att_x_em = np.asarray(att_x_em, dtype=np.float32)
    whT = np.ascontiguousarray(np.asarray(Wh, dtype=np.float32).T)
    bh = np.asarray(bh, dtype=np.float32)
    Wa = np.asarray(Wa, dtype=np.float32)
    maps = []
    for c in range(N_CORES):
        sl = slice(c * Bc, (c + 1) * Bc)
        maps.append({
            "hT": np.ascontiguousarray(h[sl].T),
            "x_em": np.ascontiguousarray(x_em[sl]),
            "att_x_em": np.ascontiguousarray(att_x_em[sl]),
            "WhT": whT,
            "bh": bh,
            "Wa": Wa,
        })
    return maps


def run(h, x_em, att_x_em, Wh, bh, Wa, reps=1, **run_kwargs):
    nc = _get_nc(reps)
    maps = _in_maps(h, x_em, att_x_em, Wh, bh, Wa)
    res = run_bass_kernel_spmd(nc, maps, list(range(N_CORES)), **run_kwargs)
    att_out = np.concatenate([r["att_out"] for r in res.results], axis=0)
    alpha = np.concatenate([r["alpha"] for r in res.results], axis=0)
    return (att_out, alpha.reshape(B, 1, S)), res


def kernel(h, x_em, att_x_em, Wh, bh, Wa, ba=None, feature_h=8, feature_w=32):
    (att_out, alpha), _ = run(h, x_em, att_x_em, Wh, bh, Wa)
    return att_out, alpha


# revision 29
# speedup vs baseline: 1.0193x; 1.0193x over previous
"""Trainium2 Bass kernel for nn_Attention (additive-attention block).

Computation (B=256, Hde=E=Catt=512, S=Hf*Wf=256):
    g     = h @ Wh.T + bh                      [B, E]
    feat  = tanh(x_em + g[:, :, None, None])   [B, E, Hf, Wf]
    score = einsum('bes,e->bs', feat, Wa) + ba [B, S]   (ba cancels in softmax)
    alpha = softmax_S(score)                   [B, S]
    out   = einsum('bs,bsc->bc', alpha, att)   [B, Catt]
Returns (att_out [B, Catt], alpha [B, 1, S]).

Sharding: data-parallel over batch B across 8 NeuronCores (32 batches per
core), weights replicated; no collectives.  Gather on host.

The two big reductions run on the PE in float32r (fp32 bits, ~11-bit
mantissa multiply) which streams at 1 cycle/row for N>=256 vs 4 for fp32;
measured mini-test rel err 2.2e-4.
"""

import numpy as np

import concourse.bacc as bacc
import concourse.bass as bass  # noqa: F401
import concourse.mybir as mybir
import concourse.tile as tile
from concourse import masks
from concourse.bass_utils import run_bass_kernel_spmd

N_CORES = 8
B, Hde, E, Catt = 256, 512, 512, 512
S = 256  # Hf * Wf = 8 * 32
Bc = B // N_CORES  # 32 batches per core
P = 128
ET = E // P    # 4 tiles over E
DT = Hde // P  # 4 tiles over Hde
ST = S // P    # 2 tiles over S
F32 = mybir.dt.float32
F32R = mybir.dt.float32r
AFT = mybir.ActivationFunctionType
AX = mybir.AxisListType

USE_F32R = True

_CACHE = {}


GROUPS = (16, 16)  # batches per softmax/output group
assert sum(GROUPS) == Bc
G = max(GROUPS)


def _emit(nc, tc, hT_d, x_d, att_d, whT_d, bh_d, wa_d, out_d, alpha_d):
    FMM = F32R if USE_F32R else F32

    def mmcast(ap):
        return ap.bitcast(F32R) if USE_F32R else ap

    with (
        tc.tile_pool(name="const", bufs=1) as cpool,
        tc.tile_pool(name="xin", bufs=5) as xpool,
        tc.tile_pool(name="attres", bufs=Bc) as apool,
        tc.tile_pool(name="feat", bufs=4) as fpool,
        tc.tile_pool(name="grp", bufs=2) as gpool,
        tc.tile_pool(name="ps", bufs=2, space="PSUM") as pspool,
        tc.tile_pool(name="psrow", bufs=3, space="PSUM") as rowpool,
    ):
        # ---- constants ----
        # e-index mapping everywhere: e = 4*p + t  (contiguous 4KB per
        # partition on the x_em DMA); s-index mapping: s = 2*p + t.
        whT_sb = cpool.tile([P, DT, E], F32)
        whT_r = whT_d[:].rearrange("(t p) e -> p t e", p=P)
        for dt in range(DT):
            nc.sync.dma_start(whT_sb[:, dt, :], whT_r[:, dt, :])
        hT_sb = cpool.tile([P, DT, Bc], F32)
        nc.sync.dma_start(hT_sb[:], hT_d[:].rearrange("(t p) b -> p t b", p=P))
        bh_sb = cpool.tile([P, ET], F32)
        nc.gpsimd.dma_start(bh_sb[:], bh_d[:].rearrange("(p t) -> p t", p=P))
        wa_sb = cpool.tile([P, ET], FMM)
        nc.gpsimd.dma_start(wa_sb[:],
                            mmcast(wa_d[:].rearrange("(p t) -> p t", p=P)))
        ident = cpool.tile([P, P], F32)
        masks.make_identity(nc, ident[:])

        # ---- gg[p, t, b] = (Wh @ h.T)[4p+t, b] + bh[4p+t] ----
        gg_sb = cpool.tile([P, ET, Bc], F32)
        for t in range(ET):
            gt_ps = pspool.tile([P, Bc], F32, tag="t32")
            for dt in range(DT):
                nc.tensor.matmul(
                    gt_ps[:],
                    lhsT=whT_sb[:, dt, t::ET],
                    rhs=hT_sb[:, dt, :],
                    start=(dt == 0), stop=(dt == DT - 1),
                )
            nc.vector.tensor_scalar_add(gg_sb[:, t, :], gt_ps[:],
                                        bh_sb[:, t:t + 1])

        att_tiles = {}
        g_state = {}

        def phase_a(g):
            Gg = GROUPS[g]
            b0 = sum(GROUPS[:g])
            scores_g = gpool.tile([Gg, S], F32, tag="scores", name=f"scores_{g}")
            for j2 in range(Gg // 2):
                sc_ps = rowpool.tile([1, 2, S], F32, tag="sc", bufs=3,
                                     name=f"sc_ps_{g}_{j2}")
                for j in range(2):
                    b = b0 + j2 * 2 + j
                    x_sb = xpool.tile([P, ET, S], F32, tag="x", name=f"x_{b}")
                    nc.sync.dma_start(
                        x_sb[:], x_d[b].rearrange("(p t) s -> p t s", p=P))
                    att_sb = apool.tile([P, ST, Catt], FMM, tag="att",
                                        name=f"att_{b}")
                    nc.sync.dma_start(
                        att_sb[:],
                        mmcast(att_d[b].rearrange("(p t) c -> p t c", p=P)))
                    att_tiles[b] = att_sb
                    for t in range(ET):
                        feat_sb = fpool.tile([P, S], FMM, tag="feat",
                                             name=f"feat_{b}_{t}")
                        nc.scalar.activation(feat_sb[:], x_sb[:, t, :], AFT.Tanh,
                                             bias=gg_sb[:, t, b:b + 1], scale=1.0)
                        nc.tensor.matmul(
                            sc_ps[0:1, j, :],
                            lhsT=wa_sb[:, t:t + 1],
                            rhs=feat_sb[:],
                            start=(t == 0), stop=(t == ET - 1),
                        )
                srow_sb = fpool.tile([1, 2, S], F32, tag="srow", bufs=4,
                                     name=f"srow_{g}_{j2}")
                nc.vector.tensor_copy(srow_sb[:], sc_ps[:])
                nc.gpsimd.dma_start(scores_g[j2 * 2:(j2 + 1) * 2, :],
                                    srow_sb[:])
            g_state[g] = scores_g

        def phase_b(g):
            Gg = GROUPS[g]
            b0 = sum(GROUPS[:g])
            last = (g == len(GROUPS) - 1)
            dma_eng = nc.sync if last else nc.gpsimd
            scores_g = g_state.pop(g)
            alpha_g = gpool.tile([Gg, S], F32, tag="alpha", name=f"alpha_{g}")
            ssum = gpool.tile([Gg, 1], F32, tag="ssum", name=f"ssum_{g}")
            nc.scalar.activation(alpha_g[:], scores_g[:], AFT.Exp,
                                 bias=0.0, scale=1.0, accum_out=ssum[:])
            rinv = gpool.tile([Gg, 1], F32, tag="rinv", name=f"rinv_{g}")
            nc.vector.reciprocal(rinv[:], ssum[:])
            nc.vector.tensor_scalar_mul(alpha_g[:], alpha_g[:], rinv[:])
            dma_eng.dma_start(alpha_d[b0:b0 + Gg, :], alpha_g[:])

            alphaT_g = gpool.tile([P, ST, Gg], FMM, tag="alphaT",
                                  name=f"alphaT_{g}")
            for t in range(ST):
                tps = pspool.tile([P, Gg], F32, tag="t32", name=f"tps_{g}_{t}")
                nc.tensor.transpose(tps[:], alpha_g[:, t::ST], ident[:Gg, :Gg])
                nc.scalar.activation(alphaT_g[:, t, :], tps[:], AFT.Identity)

            for j2 in range(Gg // 2):
                aorow_sb = fpool.tile([1, 2, Catt], F32, tag="aorow", bufs=4,
                                      name=f"aorow_{g}_{j2}")
                for j in range(2):
                    b = b0 + j2 * 2 + j
                    ao_ps = rowpool.tile([1, Catt], F32, tag="ao", bufs=3,
                                         name=f"ao_ps_{b}")
                    for t in range(ST):
                        nc.tensor.matmul(
                            ao_ps[:],
                            lhsT=alphaT_g[:, t, b - b0:b - b0 + 1],
                            rhs=att_tiles[b][:, t, :],
                            start=(t == 0), stop=(t == ST - 1),
                        )
                    if last and j == 1:
                        nc.scalar.copy(aorow_sb[0:1, j, :], ao_ps[:])
                    else:
                        nc.vector.tensor_copy(aorow_sb[0:1, j, :], ao_ps[:])
                dma_eng.dma_start(out_d[b0 + j2 * 2:b0 + (j2 + 1) * 2, :],
                                  aorow_sb[:])

        # software-pipelined emission: phase B of group g-1 is emitted inside
        # group g's phase A so the in-order PE/ACT streams never stall at a
        # group boundary.
        for g in range(len(GROUPS)):
            phase_a(g)
            if g >= 1:
                phase_b(g - 1)
        phase_b(len(GROUPS) - 1)


def _build(reps=1):
    nc = bacc.Bacc("TRN2", target_bir_lowering=False, debug=False,
                   num_devices=N_CORES)

    hT_d = nc.dram_tensor("hT", [Hde, Bc], F32, kind="ExternalInput")
    x_d = nc.dram_tensor("x_em", [Bc, E, S], F32, kind="ExternalInput")
    att_d = nc.dram_tensor("att_x_em", [Bc, S, Catt], F32, kind="ExternalInput")
    whT_d = nc.dram_tensor("WhT", [Hde, E], F32, kind="ExternalInput")
    bh_d = nc.dram_tensor("bh", [E], F32, kind="ExternalInput")
    wa_d = nc.dram_tensor("Wa", [E], F32, kind="ExternalInput")
    out_d = nc.dram_tensor("att_out", [Bc, Catt], F32, kind="ExternalOutput")
    alpha_d = nc.dram_tensor("alpha", [Bc, S], F32, kind="ExternalOutput")

    with tile.TileContext(nc) as tc:
        for _rep in range(reps):
            _emit(nc, tc, hT_d, x_d, att_d, whT_d, bh_d, wa_d, out_d, alpha_d)

    nc.compile()
    return nc


def _get_nc(reps=1):
    key = ("nc", reps)
    if key not in _CACHE:
        _CACHE[key] = _build(reps)
    return _CACHE[key]


def _in_maps(h, x_em, att_x_em, Wh, bh, Wa):
    h = np.asarray(h, dtype=np.float32)
    x_em = np.asarray(x_em, dtype=np.float32).reshape(B, E, S)
    att_x_em = np.asarray(att_x_em, dtype=np.float32)
    whT = np.ascontiguousarray(np.asarray(Wh, dtype=np.float32).T)
    bh = np.asarray(bh, dtype=np.float32)
    Wa = np.asarray(Wa, dtype=np.float32)
    maps = []
    for c in range(N_CORES):
        sl = slice(c * Bc, (c + 1) * Bc)
        maps.append({
            "hT": np.ascontiguousarray(h[sl].T),
            "x_em": np.ascontiguousarray(x_em[sl]),
            "att_x_em": np.ascontiguousarray(att_x_em[sl]),
            "WhT": whT,
            "bh": bh,
            "Wa": Wa,
        })
    return maps


def run(h, x_em, att_x_em, Wh, bh, Wa, reps=1, **run_kwargs):
    nc = _get_nc(reps)
    maps = _in_maps(h, x_em, att_x_em, Wh, bh, Wa)
    res = run_bass_kernel_spmd(nc, maps, list(range(N_CORES)), **run_kwargs)
    att_out = np.concatenate([r["att_out"] for r in res.results], axis=0)
    alpha = np.concatenate([r["alpha"] for r in res.results], axis=0)
    return (att_out, alpha.reshape(B, 1, S)), res


def kernel(h, x_em, att_x_em, Wh, bh, Wa, ba=None, feature_h=8, feature_w=32):
    (att_out, alpha), _ = run(h, x_em, att_x_em, Wh, bh, Wa)
    return att_out, alpha


# revision 31
# speedup vs baseline: 1.1389x; 1.1174x over previous
"""Trainium2 Bass kernel for nn_Attention (additive-attention block).

Computation (B=256, Hde=E=Catt=512, S=Hf*Wf=256):
    g     = h @ Wh.T + bh                      [B, E]
    feat  = tanh(x_em + g[:, :, None, None])   [B, E, Hf, Wf]
    score = einsum('bes,e->bs', feat, Wa) + ba [B, S]   (ba cancels in softmax)
    alpha = softmax_S(score)                   [B, S]
    out   = einsum('bs,bsc->bc', alpha, att)   [B, Catt]
Returns (att_out [B, Catt], alpha [B, 1, S]).

Sharding: data-parallel over batch B across 8 NeuronCores (32 batches per
core), weights replicated; no collectives.  Gather on host.

The two big reductions run on the PE in float32r (fp32 bits, ~11-bit
mantissa multiply) which streams at 1 cycle/row for N>=256 vs 4 for fp32;
measured mini-test rel err 2.2e-4.
"""

import numpy as np

import concourse.bacc as bacc
import concourse.bass as bass  # noqa: F401
import concourse.mybir as mybir
import concourse.tile as tile
from concourse import masks
from concourse.bass_utils import run_bass_kernel_spmd

N_CORES = 8
B, Hde, E, Catt = 256, 512, 512, 512
S = 256  # Hf * Wf = 8 * 32
Bc = B // N_CORES  # 32 batches per core
P = 128
ET = E // P    # 4 tiles over E
DT = Hde // P  # 4 tiles over Hde
ST = S // P    # 2 tiles over S
F32 = mybir.dt.float32
F32R = mybir.dt.float32r
AFT = mybir.ActivationFunctionType
AX = mybir.AxisListType

USE_F32R = True

_CACHE = {}


GROUPS = (16, 16)  # batches per softmax/output group
assert sum(GROUPS) == Bc
G = max(GROUPS)


def _emit(nc, tc, hT_d, x_d, att_d, whT_d, bh_d, wa_d, out_d, alpha_d):
    FMM = F32R if USE_F32R else F32

    def mmcast(ap):
        return ap.bitcast(F32R) if USE_F32R else ap

    with (
        tc.tile_pool(name="const", bufs=1) as cpool,
        tc.tile_pool(name="xin", bufs=5) as xpool,
        tc.tile_pool(name="attres", bufs=Bc) as apool,
        tc.tile_pool(name="feat", bufs=4) as fpool,
        tc.tile_pool(name="grp", bufs=2) as gpool,
        tc.tile_pool(name="ps", bufs=2, space="PSUM") as pspool,
        tc.tile_pool(name="psrow", bufs=3, space="PSUM") as rowpool,
    ):
        # ---- constants ----
        # e-index mapping everywhere: e = 4*p + t  (contiguous 4KB per
        # partition on the x_em DMA); s-index mapping: s = 2*p + t.
        whT_sb = cpool.tile([P, DT, E], F32)
        whT_r = whT_d[:].rearrange("(t p) e -> p t e", p=P)
        for dt in range(DT):
            nc.sync.dma_start(whT_sb[:, dt, :], whT_r[:, dt, :])
        hT_sb = cpool.tile([P, DT, Bc], F32)
        nc.sync.dma_start(hT_sb[:], hT_d[:].rearrange("(t p) b -> p t b", p=P))
        bh_sb = cpool.tile([P, ET], F32)
        nc.gpsimd.dma_start(bh_sb[:], bh_d[:].rearrange("(p t) -> p t", p=P))
        wa_sb = cpool.tile([P, ET], FMM)
        nc.gpsimd.dma_start(wa_sb[:],
                            mmcast(wa_d[:].rearrange("(p t) -> p t", p=P)))
        ident = cpool.tile([P, P], F32)
        masks.make_identity(nc, ident[:])

        # ---- gg[p, t, b] = (Wh @ h.T)[4p+t, b] + bh[4p+t] ----
        gg_sb = cpool.tile([P, ET, Bc], F32)
        for t in range(ET):
            gt_ps = pspool.tile([P, Bc], F32, tag="t32")
            for dt in range(DT):
                nc.tensor.matmul(
                    gt_ps[:],
                    lhsT=whT_sb[:, dt, t::ET],
                    rhs=hT_sb[:, dt, :],
                    start=(dt == 0), stop=(dt == DT - 1),
                )
            nc.vector.tensor_scalar_add(gg_sb[:, t, :], gt_ps[:],
                                        bh_sb[:, t:t + 1])

        att_tiles = {}
        g_state = {}

        def phase_a(g):
            Gg = GROUPS[g]
            b0 = sum(GROUPS[:g])
            for j2 in range(Gg // 2):
                sc_ps = rowpool.tile([1, 2, S], F32, tag="sc", bufs=2,
                                     name=f"sc_ps_{g}_{j2}")
                for j in range(2):
                    b = b0 + j2 * 2 + j
                    x_sb = xpool.tile([P, ET, S], F32, tag="x", name=f"x_{b}")
                    nc.sync.dma_start(
                        x_sb[:], x_d[b].rearrange("(p t) s -> p t s", p=P))
                    att_sb = apool.tile([P, ST, Catt], FMM, tag="att",
                                        name=f"att_{b}")
                    nc.sync.dma_start(
                        att_sb[:],
                        mmcast(att_d[b].rearrange("(p t) c -> p t c", p=P)))
                    att_tiles[b] = att_sb
                    for t in range(ET):
                        feat_sb = fpool.tile([P, S], FMM, tag="feat",
                                             name=f"feat_{b}_{t}")
                        nc.scalar.activation(feat_sb[:], x_sb[:, t, :], AFT.Tanh,
                                             bias=gg_sb[:, t, b:b + 1], scale=1.0)
                        nc.tensor.matmul(
                            sc_ps[0:1, j, :],
                            lhsT=wa_sb[:, t:t + 1],
                            rhs=feat_sb[:],
                            start=(t == 0), stop=(t == ET - 1),
                        )
                # per-pair softmax straight from PSUM (no gather):
                # exp per row (bias-free), sum+recip on DVE, normalize rows
                arow = fpool.tile([1, 2, S], F32, tag="arow", bufs=4,
                                  name=f"arow_{g}_{j2}")
                for j in range(2):
                    nc.scalar.activation(arow[0:1, j, :], sc_ps[0:1, j, :],
                                         AFT.Exp, bias=0.0, scale=1.0)
                s2 = fpool.tile([1, 2, 1], F32, tag="s2", bufs=4,
                                name=f"s2_{g}_{j2}")
                nc.vector.reduce_sum(s2[:], arow[:], axis=AX.X)
                r2 = fpool.tile([1, 2, 1], F32, tag="r2", bufs=4,
                                name=f"r2_{g}_{j2}")
                nc.vector.reciprocal(r2[:], s2[:])
                for j in range(2):
                    nc.vector.tensor_scalar_mul(arow[0:1, j, :],
                                                arow[0:1, j, :],
                                                r2[0:1, j, :])
                b2 = b0 + j2 * 2
                nc.gpsimd.dma_start(alpha_d[b2:b2 + 2, :], arow[:])

                # alphaT columns for this pair via per-row PE transposes
                alphaT2 = fpool.tile([P, ST, 2], FMM, tag="alphaT2", bufs=4,
                                     name=f"alphaT2_{g}_{j2}")
                for j in range(2):
                    for t in range(ST):
                        tps = rowpool.tile([P, 1], F32, tag="tp1", bufs=2,
                                           name=f"tp1_{g}_{j2}_{j}_{t}")
                        nc.tensor.transpose(tps[:], arow[0:1, j, t::ST],
                                            ident[:1, :1])
                        nc.vector.tensor_copy(alphaT2[:, t, j:j + 1], tps[:])

                # phase B for the pair
                aorow_sb = fpool.tile([1, 2, Catt], F32, tag="aorow", bufs=4,
                                      name=f"aorow_{g}_{j2}")
                for j in range(2):
                    b = b2 + j
                    ao_ps = rowpool.tile([1, Catt], F32, tag="ao", bufs=2,
                                         name=f"ao_ps_{b}")
                    for t in range(ST):
                        nc.tensor.matmul(
                            ao_ps[:],
                            lhsT=alphaT2[:, t, j:j + 1],
                            rhs=att_tiles[b][:, t, :],
                            start=(t == 0), stop=(t == ST - 1),
                        )
                    nc.vector.tensor_copy(aorow_sb[0:1, j, :], ao_ps[:])
                nc.gpsimd.dma_start(out_d[b2:b2 + 2, :], aorow_sb[:])

        # fully per-pair pipeline: no group barriers remain
        for g in range(len(GROUPS)):
            phase_a(g)


def _build(reps=1):
    nc = bacc.Bacc("TRN2", target_bir_lowering=False, debug=False,
                   num_devices=N_CORES)

    hT_d = nc.dram_tensor("hT", [Hde, Bc], F32, kind="ExternalInput")
    x_d = nc.dram_tensor("x_em", [Bc, E, S], F32, kind="ExternalInput")
    att_d = nc.dram_tensor("att_x_em", [Bc, S, Catt], F32, kind="ExternalInput")
    whT_d = nc.dram_tensor("WhT", [Hde, E], F32, kind="ExternalInput")
    bh_d = nc.dram_tensor("bh", [E], F32, kind="ExternalInput")
    wa_d = nc.dram_tensor("Wa", [E], F32, kind="ExternalInput")
    out_d = nc.dram_tensor("att_out", [Bc, Catt], F32, kind="ExternalOutput")
    alpha_d = nc.dram_tensor("alpha", [Bc, S], F32, kind="ExternalOutput")

    with tile.TileContext(nc) as tc:
        for _rep in range(reps):
            _emit(nc, tc, hT_d, x_d, att_d, whT_d, bh_d, wa_d, out_d, alpha_d)

    nc.compile()
    return nc


def _get_nc(reps=1):
    key = ("nc", reps)
    if key not in _CACHE:
        _CACHE[key] = _build(reps)
    return _CACHE[key]


def _in_maps(h, x_em, att_x_em, Wh, bh, Wa):
    h = np.asarray(h, dtype=np.float32)
    x_em = np.asarray(x_em, dtype=np.float32).reshape(B, E, S)
    att_x_em = np.asarray(att_x_em, dtype=np.float32)
    whT = np.ascontiguousarray(np.asarray(Wh, dtype=np.float32).T)
    bh = np.asarray(bh, dtype=np.float32)
    Wa = np.asarray(Wa, dtype=np.float32)
    maps = []
    for c in range(N_CORES):
        sl = slice(c * Bc, (c + 1) * Bc)
        maps.append({
            "hT": np.ascontiguousarray(h[sl].T),
            "x_em": np.ascontiguousarray(x_em[sl]),
            "att_x_em": np.ascontiguousarray(att_x_em[sl]),
            "WhT": whT,
            "bh": bh,
            "Wa": Wa,
        })
    return maps


def run(h, x_em, att_x_em, Wh, bh, Wa, reps=1, **run_kwargs):
    nc = _get_nc(reps)
    maps = _in_maps(h, x_em, att_x_em, Wh, bh, Wa)
    res = run_bass_kernel_spmd(nc, maps, list(range(N_CORES)), **run_kwargs)
    att_out = np.concatenate([r["att_out"] for r in res.results], axis=0)
    alpha = np.concatenate([r["alpha"] for r in res.results], axis=0)
    return (att_out, alpha.reshape(B, 1, S)), res


def kernel(h, x_em, att_x_em, Wh, bh, Wa, ba=None, feature_h=8, feature_w=32):
    (att_out, alpha), _ = run(h, x_em, att_x_em, Wh, bh, Wa)
    return att_out, alpha
